# revision 1
# baseline (speedup 1.0000x reference)
"""ChebConv GNN (3 layers, K=4) on 8 Trainium2 NeuronCores.

Sharding: nodes are partitioned across the 8 cores (graph parallel). A
load-balancing permutation (LPT on in-degree) relabels nodes so every core
owns NW windows of 128 dst nodes with near-equal edge counts. Each SpMM
(lhat application) gathers source-node feature rows from a replicated
node-major table in HBM via dma_gather, segment-sums them per 128-dst
window with a one-hot matmul on the TensorEngine, and the per-core slices
are re-replicated with an AllGather between Chebyshev hops.

Compute layout is feature-major ([feature, node] in SBUF) so the dense
W-matmuls need no transposes; node-major copies for the gather tables are
produced with PE transposes on the way out.
"""

import numpy as np

# ---------------- problem constants (hardcoded per contract) ----------------
N, E = 50000, 800000
F, HID, CLS, K = 128, 128, 40, 4
P = 128
CORES = 8
NW = 50                 # dst windows per core (must be even)
SL = NW * P             # 6400 nodes per core
NPAD = CORES * SL       # 51200 padded node count
HALF = NPAD // 2        # 25600 rows per half-table (int16-indexable)


# ---------------- host preprocessing ----------------
def _lpt_windows(indeg, n_windows, cap):
    """Assign nodes to windows (cap nodes each), balancing in-degree sums.
    Returns perm: old node id -> new node id."""
    import heapq
    order = np.argsort(-indeg, kind="stable")
    heap = [(0, wi) for wi in range(n_windows)]
    heapq.heapify(heap)
    counts = np.zeros(n_windows, np.int64)
    perm = np.empty(len(indeg), np.int64)
    for old in order:
        while True:
            load, wi = heapq.heappop(heap)
            if counts[wi] < cap:
                break
        perm[old] = wi * cap + counts[wi]
        counts[wi] += 1
        if counts[wi] < cap:
            heapq.heappush(heap, (load + int(indeg[old]), wi))
    return perm


def _preprocess(edge_src, edge_dst, n, cfg):
    """Compute norm weights, node permutation, and per-core padded edge data."""
    cores, nw, p = cfg["CORES"], cfg["NW"], P
    sl = nw * p
    npad = cores * sl
    half = npad // 2

    es = np.asarray(edge_src, np.int64)
    ed = np.asarray(edge_dst, np.int64)
    deg = np.bincount(es, minlength=n).astype(np.float32)
    dinv = np.where(deg > 0, 1.0 / np.sqrt(np.maximum(deg, 1.0)), 0.0).astype(
        np.float32
    )
    wnorm = (-dinv[es] * dinv[ed]).astype(np.float32)

    indeg = np.bincount(ed, minlength=n)
    perm = _lpt_windows(indeg, cores * nw, p)  # old -> new

    nsrc = perm[es]
    ndst = perm[ed]
    core_e = ndst // sl
    win_e = (ndst % sl) // p
    dloc_e = (ndst % p).astype(np.float32)
    half_e = (nsrc >= half).astype(np.int64)
    idx_e = (nsrc - half_e * half).astype(np.int64)

    # group edges by (core, win, half)
    gkey = (core_e * nw + win_e) * 2 + half_e
    ngroups = cores * nw * 2
    order = np.argsort(gkey, kind="stable")
    gkey_s = gkey[order]
    counts = np.bincount(gkey_s, minlength=ngroups)
    starts = np.concatenate([[0], np.cumsum(counts)[:-1]])
    rank = np.arange(len(es)) - starts[gkey_s]  # position within group

    cnts = counts.reshape(cores, nw, 2)
    CA = int(np.ceil(cnts[:, :, 0].max() / p))
    CB = int(np.ceil(cnts[:, :, 1].max() / p))
    CA = max(CA, 1)
    CB = max(CB, 1)
    CW = CA + CB

    # padded edge slot arrays
    capa = {0: CA * p, 1: CB * p}
    idx_pad = {h: np.zeros((cores, nw, capa[h]), np.int16) for h in (0, 1)}
    dl_pad = np.zeros((cores, nw, CW, p), np.float32)
    w_pad = np.zeros((cores, nw, CW, p), np.float32)

    ce, we, he = core_e[order], win_e[order], half_e[order]
    de, wne, ie = dloc_e[order], wnorm[order], idx_e[order]
    for h in (0, 1):
        m = he == h
        idx_pad[h][ce[m], we[m], rank[m]] = ie[m].astype(np.int16)
        coff = rank[m] // p + (0 if h == 0 else CA)
        dl_pad[ce[m], we[m], coff, rank[m] % p] = de[m]
        w_pad[ce[m], we[m], coff, rank[m] % p] = wne[m]

    # dma_gather index arrays per pair of windows: [cores, nw//2, 128, len/16]
    def wrap(idxs):  # idxs: [cores, nw//2, L] -> [cores, nw//2, 128, L//16]
        c, g, L = idxs.shape
        a = idxs.reshape(c, g, L // 16, 16).transpose(0, 1, 3, 2)  # [c,g,16,L/16]
        return np.tile(a, (1, 1, 8, 1)).copy()  # [c,g,128,L/16]

    idxA = wrap(idx_pad[0].reshape(cores, nw // 2, 2 * CA * p))
    idxB = wrap(idx_pad[1].reshape(cores, nw // 2, 2 * CB * p))

    # dl/w arrays in SBUF layout [cores, 128(p), nw*CW]
    dl_arr = dl_pad.transpose(0, 3, 1, 2).reshape(cores, p, nw * CW).copy()
    w_arr = w_pad.transpose(0, 3, 1, 2).reshape(cores, p, nw * CW).copy()

    return dict(
        perm=perm, wnorm=wnorm, CA=CA, CB=CB, CW=CW,
        idxA=idxA, idxB=idxB, dl=dl_arr, w=w_arr, w2=(2.0 * w_arr),
    )


# ---------------- device kernel ----------------
def _build(cfg, CA, CB):
    import concourse.bass as bass
    import concourse.bacc as bacc
    import concourse.tile as tile
    import concourse.mybir as mybir
    import dataclasses

    cores, nw = cfg["CORES"], cfg["NW"]
    sl = nw * P
    npad = cores * sl
    half = npad // 2
    CW = CA + CB
    fp = mybir.dt.float32
    bf = mybir.dt.bfloat16
    Alu = mybir.AluOpType
    Act = mybir.ActivationFunctionType

    nc = bacc.Bacc("TRN2", target_bir_lowering=False, debug=False,
                   num_devices=cores, num_swdge_queues=4)

    # -------- I/O --------
    xT_d = nc.dram_tensor("xT", [P, sl], fp, kind="ExternalInput")
    xfull_d = nc.dram_tensor("xfull", [npad, F], bf, kind="ExternalInput")
    idxA_d = nc.dram_tensor("idxA", [nw // 2, P, CA * 16], mybir.dt.int16,
                            kind="ExternalInput")
    idxB_d = nc.dram_tensor("idxB", [nw // 2, P, CB * 16], mybir.dt.int16,
                            kind="ExternalInput")
    dl_d = nc.dram_tensor("dl", [P, nw * CW], bf, kind="ExternalInput")
    wt_d = nc.dram_tensor("wt", [P, nw * CW], bf, kind="ExternalInput")
    wt2_d = nc.dram_tensor("wt2", [P, nw * CW], bf, kind="ExternalInput")
    w0_d = nc.dram_tensor("w0t", [P, K, HID], fp, kind="ExternalInput")
    w1_d = nc.dram_tensor("w1t", [P, K, HID], fp, kind="ExternalInput")
    w2_d = nc.dram_tensor("w2t", [P, K, CLS], fp, kind="ExternalInput")
    b0_d = nc.dram_tensor("b0", [HID, 1], fp, kind="ExternalInput")
    b1_d = nc.dram_tensor("b1", [HID, 1], fp, kind="ExternalInput")
    b2_d = nc.dram_tensor("b2", [CLS, 1], fp, kind="ExternalInput")
    iota_d = nc.dram_tensor("iota", [P, P], bf, kind="ExternalInput")
    ident_d = nc.dram_tensor("ident", [P, P], fp, kind="ExternalInput")
    out_d = nc.dram_tensor("out", [sl, CLS], fp, kind="ExternalOutput")

    def bcol(t, c):  # [128,1] column slice
        return t[:, c:c + 1]

    def bmid(ap, n):  # [128, X] -> [128, n, X], middle stride 0
        return dataclasses.replace(ap, ap=[ap.ap[0], [0, n], ap.ap[1]])

    def blast(ap, n):  # [128, X] -> [128, X, n], last stride 0
        return dataclasses.replace(ap, ap=[ap.ap[0], ap.ap[1], [0, n]])

    with tile.TileContext(nc) as tc:
        with (
            tc.tile_pool(name="const", bufs=1) as constp,
            tc.tile_pool(name="tx", bufs=3) as txp,
            tc.tile_pool(name="acc", bufs=1) as accp,
            tc.tile_pool(name="g", bufs=2) as gp,
            tc.tile_pool(name="m", bufs=2) as mp,
            tc.tile_pool(name="ix", bufs=2) as ixp,
            tc.tile_pool(name="st", bufs=4) as stp,
            tc.tile_pool(name="psA", bufs=2, space="PSUM") as psA,
            tc.tile_pool(name="psT", bufs=2, space="PSUM") as psT,
            tc.tile_pool(name="psW", bufs=2, space="PSUM") as psW,
            tc.tile_pool(name="dram", bufs=2, space="DRAM") as dramp,
            tc.tile_pool(name="tabs", bufs=3, space="DRAM") as tabp,
        ):
            # -------- constants --------
            dl_t = constp.tile([P, nw * CW], bf)
            wt_t = constp.tile([P, nw * CW], bf)
            wt2_t = constp.tile([P, nw * CW], bf)
            iota_t = constp.tile([P, P], bf)
            ident_t = constp.tile([P, P], fp)
            w0_t = constp.tile([P, K, HID], fp)
            w1_t = constp.tile([P, K, HID], fp)
            w2_t = constp.tile([P, K, CLS], fp)
            b0_t = constp.tile([HID, 1], fp)
            b1_t = constp.tile([HID, 1], fp)
            b2_t = constp.tile([CLS, 1], fp)
            for t, d in ((dl_t, dl_d), (wt_t, wt_d), (wt2_t, wt2_d),
                         (iota_t, iota_d), (ident_t, ident_d),
                         (w0_t, w0_d), (w1_t, w1_d), (w2_t, w2_d),
                         (b0_t, b0_d), (b1_t, b1_d), (b2_t, b2_d)):
                nc.sync.dma_start(out=t[:], in_=d[:])

            tx0 = txp.tile([P, sl], fp, tag="tx")
            nc.sync.dma_start(out=tx0[:], in_=xT_d[:, :])

            tabA_in = xfull_d[0:half, :]
            tabB_in = xfull_d[half:npad, :]

            def spmm(wsel_t, tabA, tabB, tx_prev2, Wt, fo, acc, k, want_slice):
                """One lhat application; returns (tx_new, slice_dram|None)."""
                tx_new = txp.tile([P, sl], fp, tag="tx")
                slice_d = (dramp.tile([sl, F], bf, tag="slice", name="slice_d")
                           if want_slice else None)
                nA, nB = 2 * CA * P, 2 * CB * P
                for g in range(nw // 2):
                    ixA = ixp.tile([P, CA * 16], mybir.dt.int16, tag="ixA")
                    nc.sync.dma_start(out=ixA[:], in_=idxA_d[g])
                    ixB = ixp.tile([P, CB * 16], mybir.dt.int16, tag="ixB")
                    nc.sync.dma_start(out=ixB[:], in_=idxB_d[g])
                    GA = gp.tile([P, 2 * CA, P], bf, tag="GA")
                    nc.gpsimd.dma_gather(
                        out_ap=GA[:], in_ap=tabA, idxs_ap=ixA[:],
                        num_idxs=nA, num_idxs_reg=nA, elem_size=P,
                        single_packet=False, queue_num=(2 * g) % 4)
                    GB = gp.tile([P, 2 * CB, P], bf, tag="GB")
                    nc.gpsimd.dma_gather(
                        out_ap=GB[:], in_ap=tabB, idxs_ap=ixB[:],
                        num_idxs=nB, num_idxs_reg=nB, elem_size=P,
                        single_packet=False, queue_num=(2 * g + 1) % 4)
                    for h in (0, 1):
                        w = 2 * g + h
                        wb = slice(w * P, (w + 1) * P)
                        colsl = slice(w * CW, (w + 1) * CW)
                        M = mp.tile([P, CW, P], bf, tag="M")
                        nc.vector.tensor_tensor(
                            out=M[:], in0=bmid(iota_t[:], CW),
                            in1=blast(dl_t[:, colsl], P), op=Alu.is_equal)
                        nc.vector.tensor_tensor(
                            out=M[:], in0=M[:],
                            in1=blast(wsel_t[:, colsl], P), op=Alu.mult)
                        ps = psA.tile([P, P], fp, tag="ps")
                        for c in range(CW):
                            Gsl = (GA[:, h * CA + c, :] if c < CA
                                   else GB[:, h * CB + (c - CA), :])
                            nc.tensor.matmul(out=ps[:], lhsT=Gsl, rhs=M[:, c, :],
                                             start=(c == 0), stop=(c == CW - 1))
                        if tx_prev2 is None:
                            nc.vector.tensor_copy(out=tx_new[:, wb], in_=ps[:])
                        else:
                            nc.vector.tensor_tensor(
                                out=tx_new[:, wb], in0=ps[:],
                                in1=tx_prev2[:, wb], op=Alu.subtract)
                        psw = psW.tile([P, P], fp, tag="psw")
                        nc.tensor.matmul(out=psw[:fo, :], lhsT=Wt[:, k, :fo],
                                         rhs=tx_new[:, wb], start=True, stop=True)
                        nc.vector.tensor_tensor(out=acc[:fo, wb], in0=acc[:fo, wb],
                                                in1=psw[:fo, :], op=Alu.add)
                        if slice_d is not None:
                            pst = psT.tile([P, P], fp, tag="pst")
                            nc.tensor.transpose(out=pst[:], in_=tx_new[:, wb],
                                                identity=ident_t[:])
                            st = stp.tile([P, P], bf, tag="st")
                            nc.scalar.copy(out=st[:], in_=pst[:])
                            nc.scalar.dma_start(out=slice_d[w * P:(w + 1) * P, :],
                                                in_=st[:])
                return tx_new, slice_d

            def allgather(slice_d):
                tab = tabp.tile([npad, F], bf, tag="tab", addr_space="Shared")
                nc.gpsimd.collective_compute(
                    "AllGather", Alu.bypass,
                    replica_groups=[list(range(cores))],
                    ins=[slice_d[:, :].opt()], outs=[tab[:, :].opt()])
                return tab

            stage = cfg.get("STAGE", 99)
            for l, (Wt, b_t, fo) in enumerate(
                    ((w0_t, b0_t, HID), (w1_t, b1_t, HID), (w2_t, b2_t, CLS))):
                if l * 10 >= stage:
                    break
                last = l == 2
                acc = accp.tile([P, sl], fp, tag="acc")
                # ---- k=0 term: acc = W[0].T @ tx0 + b ----
                for w in range(nw):
                    wb = slice(w * P, (w + 1) * P)
                    psw = psW.tile([P, P], fp, tag="psw")
                    nc.tensor.matmul(out=psw[:fo, :], lhsT=Wt[:, 0, :fo],
                                     rhs=tx0[:, wb], start=True, stop=True)
                    nc.vector.tensor_scalar(
                        out=acc[:fo, wb], in0=psw[:fo, :],
                        scalar1=b_t[:fo, 0:1], scalar2=None, op0=Alu.add)
                # ---- k=1..3 ----
                if stage < l * 10 + 2:
                    break
                tx1, sl1 = spmm(wt_t, tabA_in, tabB_in, None, Wt, fo, acc, 1,
                                stage >= l * 10 + 3)
                if stage < l * 10 + 3:
                    break
                t1 = allgather(sl1)
                if stage < l * 10 + 4:
                    break
                tx2, sl2 = spmm(wt2_t, t1[0:half, :], t1[half:npad, :], tx0,
                                Wt, fo, acc, 2, stage >= l * 10 + 5)
                if stage < l * 10 + 5:
                    break
                t2 = allgather(sl2)
                if stage < l * 10 + 6:
                    break
                tx3, _ = spmm(wt2_t, t2[0:half, :], t2[half:npad, :], tx1,
                              Wt, fo, acc, 3, False)
                if stage < l * 10 + 7:
                    break
                # ---- epilogue ----
                if not last:
                    hT = txp.tile([P, sl], fp, tag="tx")
                    slice_h = dramp.tile([sl, F], bf, tag="slice")
                    for w in range(nw):
                        wb = slice(w * P, (w + 1) * P)
                        nc.scalar.activation(out=hT[:, wb], in_=acc[:, wb],
                                             func=Act.Relu)
                        pst = psT.tile([P, P], fp, tag="pst")
                        nc.tensor.transpose(out=pst[:], in_=hT[:, wb],
                                            identity=ident_t[:])
                        st = stp.tile([P, P], bf, tag="st")
                        nc.scalar.copy(out=st[:], in_=pst[:])
                        nc.scalar.dma_start(out=slice_h[w * P:(w + 1) * P, :],
                                            in_=st[:])
                    th = allgather(slice_h)
                    tx0 = hT
                    tabA_in, tabB_in = th[0:half, :], th[half:npad, :]
                else:
                    for w in range(nw):
                        wb = slice(w * P, (w + 1) * P)
                        pst = psT.tile([P, P], fp, tag="pst")
                        nc.tensor.transpose(out=pst[:, :CLS], in_=acc[:CLS, wb],
                                            identity=ident_t[:CLS, :CLS])
                        nm = stp.tile([P, 1], fp, tag="nm")
                        nc.vector.tensor_reduce(
                            out=nm[:], in_=pst[:, :CLS], op=Alu.max,
                            axis=mybir.AxisListType.X, negate=True)
                        ex = stp.tile([P, CLS], fp, tag="ex")
                        ssum = stp.tile([P, 1], fp, tag="ssum")
                        nc.scalar.activation(out=ex[:], in_=pst[:, :CLS],
                                             func=Act.Exp, bias=nm[:, 0:1],
                                             accum_out=ssum[:, 0:1])
                        lse = stp.tile([P, 1], fp, tag="lse")
                        nc.scalar.activation(out=lse[:], in_=ssum[:], func=Act.Ln)
                        res = stp.tile([P, CLS], fp, tag="res")
                        nc.vector.tensor_scalar(
                            out=res[:], in0=pst[:, :CLS],
                            scalar1=nm[:, 0:1], scalar2=lse[:, 0:1],
                            op0=Alu.add, op1=Alu.subtract)
                        nc.scalar.dma_start(out=out_d[w * P:(w + 1) * P, :],
                                            in_=res[:])

    nc.compile()
    return nc


_CACHE = {}


def _get_nc(cfg, CA, CB):
    key = (cfg["CORES"], cfg["NW"], CA, CB, cfg.get("STAGE", 99))
    if key not in _CACHE:
        _CACHE[key] = _build(cfg, CA, CB)
    return _CACHE[key]


def _run(x, edge_src, edge_dst, W0, b0, W1, b1, W2, b2, cfg=None,
         trace=False, trace_cores=None):
    from concourse import bass_utils

    cfg = cfg or {"CORES": CORES, "NW": NW}
    cores, nw = cfg["CORES"], cfg["NW"]
    sl = nw * P
    npad = cores * sl
    n = x.shape[0]

    import ml_dtypes
    bf16 = ml_dtypes.bfloat16

    pre = _preprocess(edge_src, edge_dst, n, cfg)
    perm, CA, CB = pre["perm"], pre["CA"], pre["CB"]

    x = np.asarray(x, np.float32)
    x_pad = np.zeros((npad, F), np.float32)
    x_pad[perm] = x

    w0t = np.ascontiguousarray(np.transpose(np.asarray(W0, np.float32), (1, 0, 2)))
    w1t = np.ascontiguousarray(np.transpose(np.asarray(W1, np.float32), (1, 0, 2)))
    w2t = np.ascontiguousarray(np.transpose(np.asarray(W2, np.float32), (1, 0, 2)))
    iota = np.broadcast_to(np.arange(P, dtype=np.float32), (P, P)).copy()
    ident = np.eye(P, dtype=np.float32)

    in_maps = []
    for c in range(cores):
        rows = slice(c * sl, (c + 1) * sl)
        in_maps.append(dict(
            xT=np.ascontiguousarray(x_pad[rows].T),
            xfull=x_pad.astype(bf16),
            idxA=pre["idxA"][c], idxB=pre["idxB"][c],
            dl=pre["dl"][c].astype(bf16), wt=pre["w"][c].astype(bf16),
            wt2=pre["w2"][c].astype(bf16),
            w0t=w0t, w1t=w1t, w2t=w2t,
            b0=np.asarray(b0, np.float32).reshape(HID, 1),
            b1=np.asarray(b1, np.float32).reshape(HID, 1),
            b2=np.asarray(b2, np.float32).reshape(CLS, 1),
            iota=iota.astype(bf16), ident=ident,
        ))

    nc = _get_nc(cfg, CA, CB)
    kw = {}
    if trace:
        kw = dict(trace=True,
                  trace_cores=trace_cores if trace_cores is not None else [0])
    res = bass_utils.run_bass_kernel_spmd(nc, in_maps,
                                          core_ids=list(range(cores)), **kw)

    full = np.concatenate([res.results[c]["out"] for c in range(cores)], axis=0)
    out = full[perm]  # inverse permutation: row for old node i is at full[perm[i]]
    return out.astype(np.float32), res


def kernel(x, edge_src, edge_dst, W0, b0, W1, b1, W2, b2):
    out, _ = _run(x, edge_src, edge_dst, W0, b0, W1, b1, W2, b2)
    return out



# revision 8
# speedup vs baseline: 1.0428x; 1.0428x over previous
"""ChebConv GNN (3 layers, K=4) on 8 Trainium2 NeuronCores.

Monomial reformulation: Chebyshev polynomials are expanded so every hop is a
plain SpMM against the raw adjacency, u_{k+1} = A_hat (dinv^2 * u_k), with the
symmetric normalization folded into the node-major feature tables (dinv or
dinv^2 per row, applied with fused per-partition activation scales) and the
Chebyshev coefficients folded into the dense weights:
  out = x@V0 + diag(dinv) (u1@V1 + u2@V2 + u3@V3),
  V0 = W0-W2, V1 = 3W3-W1, V2 = 2W2, V3 = -4W3.
The scatter matrix per destination window is therefore a pure one-hot built
with a single is_equal per window pair, and the per-edge weight multiply
disappears from the inner loop entirely.

Sharding: nodes are partitioned across the 8 cores; destination nodes are
grouped into 64-wide windows (LPT-balanced on in-degree), processed in pairs.
Each hop gathers source rows from a replicated node-major table in HBM
(dma_gather, int16 indices, A/B table halves), segment-sums them with one-hot
matmuls on the TensorEngine into feature-major PSUM, and re-replicates the
per-core table slices with an AllGather between hops.
"""

import numpy as np

# ---------------- problem constants (hardcoded per contract) ----------------
N, E = 50000, 800000
F, HID, CLS, K = 128, 128, 40, 4
P = 128
CORES = 8
WW = 64                  # dst window width
NWIN = 100               # windows per core
NPAIR = NWIN // 2        # window pairs per core
SL = NWIN * WW           # 6400 nodes per core
NPAD = CORES * SL        # 51200 padded node count
HALF = NPAD // 2         # 25600 rows per half-table (int16-indexable)


# ---------------- host preprocessing ----------------
def _lpt_windows(indeg, n_windows, cap):
    """Assign nodes to windows (cap nodes each), balancing in-degree sums.
    Returns perm: old node id -> new node id."""
    import heapq
    order = np.argsort(-indeg, kind="stable")
    heap = [(0, wi) for wi in range(n_windows)]
    heapq.heapify(heap)
    counts = np.zeros(n_windows, np.int64)
    perm = np.empty(len(indeg), np.int64)
    for old in order:
        while True:
            load, wi = heapq.heappop(heap)
            if counts[wi] < cap:
                break
        perm[old] = wi * cap + counts[wi]
        counts[wi] += 1
        if counts[wi] < cap:
            heapq.heappush(heap, (load + int(indeg[old]), wi))
    return perm


def _preprocess(edge_src, edge_dst, n):
    """Node permutation, per-core padded gather indices, and dl arrays."""
    es = np.asarray(edge_src, np.int64)
    ed = np.asarray(edge_dst, np.int64)
    deg = np.bincount(es, minlength=n).astype(np.float32)
    dinv = np.where(deg > 0, 1.0 / np.sqrt(np.maximum(deg, 1.0)), 0.0).astype(
        np.float32
    )

    indeg = np.bincount(ed, minlength=n)
    perm = _lpt_windows(indeg, CORES * NWIN, WW)  # old -> new

    nsrc = perm[es]
    ndst = perm[ed]
    core_e = ndst // SL
    win_e = (ndst % SL) // WW          # 0..NWIN-1
    dloc_e = (ndst % WW).astype(np.float32)
    half_e = (nsrc >= HALF).astype(np.int64)
    idx_e = (nsrc - half_e * HALF).astype(np.int64)

    # group edges by (core, win, half)
    gkey = (core_e * NWIN + win_e) * 2 + half_e
    ngroups = CORES * NWIN * 2
    order = np.argsort(gkey, kind="stable")
    gkey_s = gkey[order]
    counts = np.bincount(gkey_s, minlength=ngroups)
    starts = np.concatenate([[0], np.cumsum(counts)[:-1]])
    rank = np.arange(len(es)) - starts[gkey_s]  # position within group

    cnts = counts.reshape(CORES, NWIN, 2)
    CA = max(int(np.ceil(cnts[:, :, 0].max() / P)), 1)
    CB = max(int(np.ceil(cnts[:, :, 1].max() / P)), 1)
    CWW = CA + CB                     # slot blocks per window

    capa = {0: CA * P, 1: CB * P}
    # idx slot arrays: pad with 0 (valid row; dl=-1 zeroes contribution)
    idx_pad = {h: np.zeros((CORES, NWIN, capa[h]), np.int16) for h in (0, 1)}
    # dl: [cores, 128, NWIN*CWW], block order per window: A blocks then B
    dl_pad = np.full((CORES, NWIN, CWW, P), -1.0, np.float32)

    ce, we, he = core_e[order], win_e[order], half_e[order]
    de, ie = dloc_e[order], idx_e[order]
    for h in (0, 1):
        m = he == h
        idx_pad[h][ce[m], we[m], rank[m]] = ie[m].astype(np.int16)
        boff = rank[m] // P + (0 if h == 0 else CA)
        dl_pad[ce[m], we[m], boff, rank[m] % P] = de[m]

    # dma_gather index arrays per pair: linear order = [win2g blocks, win2g+1]
    def wrap(idxs):  # [cores, NPAIR, L] -> [cores, 128, NPAIR*(L//16)]
        c, g, L = idxs.shape
        a = idxs.reshape(c, g, L // 16, 16).transpose(0, 1, 3, 2)  # [c,g,16,L/16]
        a = np.tile(a, (1, 1, 8, 1))  # [c,g,128,L/16]
        return np.ascontiguousarray(a.transpose(0, 2, 1, 3).reshape(c, P, -1))

    idxA = wrap(idx_pad[0].reshape(CORES, NPAIR, 2 * CA * P))
    idxB = wrap(idx_pad[1].reshape(CORES, NPAIR, 2 * CB * P))

    # dl in SBUF layout [cores, 128(p), NWIN*CWW]
    dl_arr = np.ascontiguousarray(
        dl_pad.transpose(0, 3, 1, 2).reshape(CORES, P, NWIN * CWW))

    # node-major dinv packed per pair: [cores, 128, NPAIR]
    dn = np.zeros(NPAD, np.float32)
    dn[perm] = dinv
    dn_c = dn.reshape(CORES, NPAIR, P)          # pair g rows = 128 nodes
    dinv_nm = np.ascontiguousarray(dn_c.transpose(0, 2, 1))
    dinv2_nm = np.ascontiguousarray((dn_c ** 2).transpose(0, 2, 1))

    return dict(perm=perm, dinv=dinv, CA=CA, CB=CB,
                idxA=idxA, idxB=idxB, dl=dl_arr,
                dinv_nm=dinv_nm, dinv2_nm=dinv2_nm)


# ---------------- device kernel ----------------
def _build(CA, CB, stage=99):
    import concourse.bass as bass
    import concourse.bacc as bacc
    import concourse.tile as tile
    import concourse.mybir as mybir
    import dataclasses

    CWW = CA + CB
    NBLK = NWIN * CWW
    fp = mybir.dt.float32
    bf = mybir.dt.bfloat16
    Alu = mybir.AluOpType
    Act = mybir.ActivationFunctionType

    nc = bacc.Bacc("TRN2", target_bir_lowering=False, debug=False,
                   num_devices=CORES, num_swdge_queues=4)

    # -------- I/O --------
    xfm_d = nc.dram_tensor("xfm", [P, SL], bf, kind="ExternalInput")
    xtab_d = nc.dram_tensor("xtab", [NPAD, F], bf, kind="ExternalInput")
    idxA_d = nc.dram_tensor("idxA", [P, NPAIR * CA * 16], mybir.dt.int16,
                            kind="ExternalInput")
    idxB_d = nc.dram_tensor("idxB", [P, NPAIR * CB * 16], mybir.dt.int16,
                            kind="ExternalInput")
    dl_d = nc.dram_tensor("dl", [P, NBLK], bf, kind="ExternalInput")
    dinv_d = nc.dram_tensor("dinv", [P, NPAIR], fp, kind="ExternalInput")
    dinv2_d = nc.dram_tensor("dinv2", [P, NPAIR], fp, kind="ExternalInput")
    v0_d = nc.dram_tensor("v0", [P, K, HID], bf, kind="ExternalInput")
    v1_d = nc.dram_tensor("v1", [P, K, HID], bf, kind="ExternalInput")
    v2_d = nc.dram_tensor("v2", [P, K, CLS], bf, kind="ExternalInput")
    b0_d = nc.dram_tensor("b0", [HID, 1], fp, kind="ExternalInput")
    b1_d = nc.dram_tensor("b1", [HID, 1], fp, kind="ExternalInput")
    b2_d = nc.dram_tensor("b2", [CLS, 1], fp, kind="ExternalInput")
    iota_d = nc.dram_tensor("iota", [P, WW], bf, kind="ExternalInput")
    identb_d = nc.dram_tensor("identb", [P, P], bf, kind="ExternalInput")
    identf_d = nc.dram_tensor("identf", [P, P], fp, kind="ExternalInput")
    out_d = nc.dram_tensor("out", [SL, CLS], fp, kind="ExternalOutput")

    def bmid(ap, n):  # [128, X] -> [128, n, X], middle stride 0
        return dataclasses.replace(ap, ap=[ap.ap[0], [0, n], ap.ap[1]])

    def blast(ap, n):  # [128, X] -> [128, X, n], last stride 0
        return dataclasses.replace(ap, ap=[ap.ap[0], ap.ap[1], [0, n]])

    with tile.TileContext(nc) as tc:
        with (
            tc.tile_pool(name="const", bufs=1) as constp,
            tc.tile_pool(name="h", bufs=1) as hp,
            tc.tile_pool(name="acc", bufs=1) as accp,
            tc.tile_pool(name="g", bufs=3) as gp,
            tc.tile_pool(name="m", bufs=3) as mp,
            tc.tile_pool(name="u", bufs=3) as up,
            tc.tile_pool(name="st", bufs=3) as stp,
            tc.tile_pool(name="ep", bufs=3) as epp,
            tc.tile_pool(name="psS", bufs=2, space="PSUM") as psS,
            tc.tile_pool(name="psW", bufs=2, space="PSUM") as psW,
            tc.tile_pool(name="psT", bufs=2, space="PSUM") as psT,
            tc.tile_pool(name="dram", bufs=2, space="DRAM") as dramp,
            tc.tile_pool(name="tabs", bufs=3, space="DRAM") as tabp,
        ):
            # -------- constants --------
            dl_t = constp.tile([P, NBLK], bf)
            dinv_t = constp.tile([P, NPAIR], fp)
            dinv2_t = constp.tile([P, NPAIR], fp)
            iota_t = constp.tile([P, WW], bf)
            identb_t = constp.tile([P, P], bf)
            identf_t = constp.tile([P, P], fp)
            idxA_t = constp.tile([P, NPAIR * CA * 16], mybir.dt.int16)
            idxB_t = constp.tile([P, NPAIR * CB * 16], mybir.dt.int16)
            v0_t = constp.tile([P, K, HID], bf)
            v1_t = constp.tile([P, K, HID], bf)
            v2_t = constp.tile([P, K, CLS], bf)
            b0_t = constp.tile([HID, 1], fp)
            b1_t = constp.tile([HID, 1], fp)
            b2_t = constp.tile([CLS, 1], fp)
            for t, d in ((dl_t, dl_d), (dinv_t, dinv_d), (dinv2_t, dinv2_d),
                         (iota_t, iota_d), (identb_t, identb_d),
                         (identf_t, identf_d), (idxA_t, idxA_d),
                         (idxB_t, idxB_d), (v0_t, v0_d), (v1_t, v1_d),
                         (v2_t, v2_d), (b0_t, b0_d), (b1_t, b1_d),
                         (b2_t, b2_d)):
                nc.sync.dma_start(out=t[:], in_=d[:])

            h_fm = hp.tile([P, SL], bf, tag="hfm")
            nc.sync.dma_start(out=h_fm[:], in_=xfm_d[:, :])

            def hop(tabA, tabB, Vt, k, fo, acc_b, want_table):
                """u_{k} = A_hat @ table; acc_b (+)= u_k @ V_k; optionally
                write dinv^2-scaled node-major slices for the next table."""
                slice_d = (dramp.tile([SL, F], bf, tag="slice",
                                      name="slice_d")
                           if want_table else None)
                nA, nB = 2 * CA * P, 2 * CB * P
                for g in range(NPAIR):
                    GA = gp.tile([P, 2 * CA, P], bf, tag="GA")
                    nc.gpsimd.dma_gather(
                        out_ap=GA[:], in_ap=tabA,
                        idxs_ap=idxA_t[:, g * CA * 16:(g + 1) * CA * 16],
                        num_idxs=nA, num_idxs_reg=nA, elem_size=P,
                        single_packet=False, queue_num=(2 * g) % 4)
                    GB = gp.tile([P, 2 * CB, P], bf, tag="GB")
                    nc.gpsimd.dma_gather(
                        out_ap=GB[:], in_ap=tabB,
                        idxs_ap=idxB_t[:, g * CB * 16:(g + 1) * CB * 16],
                        num_idxs=nB, num_idxs_reg=nB, elem_size=P,
                        single_packet=False, queue_num=(2 * g + 1) % 4)
                    M = mp.tile([P, 2 * CWW, WW], bf, tag="M")
                    nc.vector.tensor_tensor(
                        out=M[:], in0=bmid(iota_t[:], 2 * CWW),
                        in1=blast(dl_t[:, 2 * g * CWW:(2 * g + 2) * CWW], WW),
                        op=Alu.is_equal)
                    u_pair = up.tile([P, P], bf, tag="u")
                    for h2 in (0, 1):
                        ps = psS.tile([P, WW], fp, tag="ps")
                        for b in range(CWW):
                            Gsl = (GA[:, h2 * CA + b, :] if b < CA
                                   else GB[:, h2 * CB + (b - CA), :])
                            nc.tensor.matmul(out=ps[:], lhsT=Gsl,
                                             rhs=M[:, h2 * CWW + b, :],
                                             start=(b == 0),
                                             stop=(b == CWW - 1))
                        nc.scalar.activation(
                            out=u_pair[:, h2 * WW:(h2 + 1) * WW], in_=ps[:],
                            func=Act.Copy)
                    pb = slice(2 * g * WW, (2 * g + 2) * WW)
                    psw = psW.tile([P, P], fp, tag="psw")
                    nc.tensor.matmul(out=psw[:fo, :], lhsT=Vt[:, k, :fo],
                                     rhs=u_pair[:], start=True, stop=True)
                    if k == 1:
                        nc.vector.tensor_copy(out=acc_b[:fo, pb],
                                              in_=psw[:fo, :])
                    else:
                        nc.vector.tensor_tensor(out=acc_b[:fo, pb],
                                                in0=acc_b[:fo, pb],
                                                in1=psw[:fo, :], op=Alu.add)
                    if want_table:
                        pst = psT.tile([P, P], bf, tag="pstb")
                        nc.tensor.transpose(out=pst[:], in_=u_pair[:],
                                            identity=identb_t[:])
                        st = stp.tile([P, F], bf, tag="st")
                        nc.scalar.activation(out=st[:], in_=pst[:],
                                             func=Act.Copy,
                                             scale=dinv2_t[:, g:g + 1])
                        nc.sync.dma_start(out=slice_d[g * P:(g + 1) * P, :],
                                          in_=st[:])
                return slice_d

            def allgather(slice_d):
                tab = tabp.tile([NPAD, F], bf, tag="tab", addr_space="Shared")
                nc.gpsimd.collective_compute(
                    "AllGather", Alu.bypass,
                    replica_groups=[list(range(CORES))],
                    ins=[slice_d[:, :].opt()], outs=[tab[:, :].opt()])
                return tab[0:HALF, :], tab[HALF:NPAD, :]

            tabA_in, tabB_in = xtab_d[0:HALF, :], xtab_d[HALF:NPAD, :]
            for l, (Vt, b_t, fo) in enumerate(
                    ((v0_t, b0_t, HID), (v1_t, b1_t, HID), (v2_t, b2_t, CLS))):
                if l * 10 >= stage:
                    break
                last = l == 2
                acc_a = accp.tile([P, SL], fp, tag="acca")
                acc_b = accp.tile([P, SL], fp, tag="accb")
                # ---- k=0 term: acc_a = V0.T @ h + b ----
                for g in range(NPAIR):
                    pb = slice(2 * g * WW, (2 * g + 2) * WW)
                    psw = psW.tile([P, P], fp, tag="psw")
                    nc.tensor.matmul(out=psw[:fo, :], lhsT=Vt[:, 0, :fo],
                                     rhs=h_fm[:, pb], start=True, stop=True)
                    nc.scalar.activation(out=acc_a[:fo, pb], in_=psw[:fo, :],
                                         func=Act.Identity, bias=b_t[:fo, 0:1])
                # ---- hops 1..3 ----
                if stage < l * 10 + 2:
                    break
                s1 = hop(tabA_in, tabB_in, Vt, 1, fo, acc_b, True)
                if stage < l * 10 + 3:
                    break
                tA, tB = allgather(s1)
                if stage < l * 10 + 4:
                    break
                s2 = hop(tA, tB, Vt, 2, fo, acc_b, True)
                if stage < l * 10 + 5:
                    break
                tA, tB = allgather(s2)
                if stage < l * 10 + 6:
                    break
                hop(tA, tB, Vt, 3, fo, acc_b, False)
                if stage < l * 10 + 7:
                    break
                # ---- epilogue: out_nm = acc_a + dinv*acc_b per pair ----
                if not last:
                    slice_h = dramp.tile([SL, F], bf, tag="slice")
                    for g in range(NPAIR):
                        pb = slice(2 * g * WW, (2 * g + 2) * WW)
                        pa = psT.tile([P, P], fp, tag="pst")
                        nc.tensor.transpose(out=pa[:], in_=acc_a[:, pb],
                                            identity=identf_t[:])
                        pbt = psT.tile([P, P], fp, tag="pst")
                        nc.tensor.transpose(out=pbt[:], in_=acc_b[:, pb],
                                            identity=identf_t[:])
                        hn = epp.tile([P, P], fp, tag="hn")
                        nc.vector.tensor_scalar(
                            out=hn[:], in0=pbt[:], scalar1=dinv_t[:, g:g + 1],
                            scalar2=None, op0=Alu.mult)
                        nc.vector.tensor_tensor(out=hn[:], in0=hn[:],
                                                in1=pa[:], op=Alu.add)
                        # table slice = dinv * relu(hn)
                        sth = stp.tile([P, F], bf, tag="st")
                        nc.scalar.activation(out=sth[:], in_=hn[:],
                                             func=Act.Relu,
                                             scale=dinv_t[:, g:g + 1])
                        nc.sync.dma_start(out=slice_h[g * P:(g + 1) * P, :],
                                          in_=sth[:])
                        # h (feature-major) = relu(hn) transposed back
                        hb = epp.tile([P, P], bf, tag="hb")
                        nc.scalar.activation(out=hb[:], in_=hn[:],
                                             func=Act.Relu)
                        ph = psT.tile([P, P], bf, tag="pstb")
                        nc.tensor.transpose(out=ph[:], in_=hb[:],
                                            identity=identb_t[:])
                        nc.scalar.activation(out=h_fm[:, pb], in_=ph[:],
                                             func=Act.Copy)
                    tabA_in, tabB_in = allgather(slice_h)
                else:
                    for g in range(NPAIR):
                        pb = slice(2 * g * WW, (2 * g + 2) * WW)
                        pa = psT.tile([P, P], fp, tag="pst")
                        nc.tensor.transpose(out=pa[:, :CLS],
                                            in_=acc_a[:CLS, pb],
                                            identity=identf_t[:CLS, :CLS])
                        pbt = psT.tile([P, P], fp, tag="pst")
                        nc.tensor.transpose(out=pbt[:, :CLS],
                                            in_=acc_b[:CLS, pb],
                                            identity=identf_t[:CLS, :CLS])
                        t = epp.tile([P, CLS], fp, tag="t")
                        nc.vector.tensor_scalar(
                            out=t[:], in0=pbt[:, :CLS],
                            scalar1=dinv_t[:, g:g + 1],
                            scalar2=None, op0=Alu.mult)
                        nc.vector.tensor_tensor(out=t[:], in0=t[:],
                                                in1=pa[:, :CLS], op=Alu.add)
                        nm = epp.tile([P, 1], fp, tag="nm")
                        nc.vector.tensor_reduce(
                            out=nm[:], in_=t[:], op=Alu.max,
                            axis=mybir.AxisListType.X, negate=True)
                        ex = epp.tile([P, CLS], fp, tag="ex")
                        ssum = epp.tile([P, 1], fp, tag="ssum")
                        nc.scalar.activation(out=ex[:], in_=t[:],
                                             func=Act.Exp, bias=nm[:, 0:1],
                                             accum_out=ssum[:, 0:1])
                        lse = epp.tile([P, 1], fp, tag="lse")
                        nc.scalar.activation(out=lse[:], in_=ssum[:],
                                             func=Act.Ln)
                        res = epp.tile([P, CLS], fp, tag="res")
                        nc.vector.tensor_scalar(
                            out=res[:], in0=t[:],
                            scalar1=nm[:, 0:1], scalar2=lse[:, 0:1],
                            op0=Alu.add, op1=Alu.subtract)
                        nc.sync.dma_start(out=out_d[g * P:(g + 1) * P, :],
                                          in_=res[:])

    nc.compile()
    return nc


_CACHE = {}


def _get_nc(CA, CB, stage=99):
    key = (CA, CB, stage)
    if key not in _CACHE:
        _CACHE[key] = _build(CA, CB, stage)
    return _CACHE[key]


def _run(x, edge_src, edge_dst, W0, b0, W1, b1, W2, b2,
         trace=False, trace_cores=None, stage=99):
    from concourse import bass_utils

    import ml_dtypes
    bf16 = ml_dtypes.bfloat16

    n = x.shape[0]
    pre = _preprocess(edge_src, edge_dst, n)
    perm, dinv, CA, CB = pre["perm"], pre["dinv"], pre["CA"], pre["CB"]

    x = np.asarray(x, np.float32)
    x_pad = np.zeros((NPAD, F), np.float32)
    x_pad[perm] = x
    xtab = np.zeros((NPAD, F), np.float32)
    xtab[perm] = dinv[:, None] * x

    # folded monomial weights
    def fold(W):
        W = np.asarray(W, np.float32)
        V = np.stack([W[0] - W[2], 3.0 * W[3] - W[1], 2.0 * W[2],
                      -4.0 * W[3]])
        return np.ascontiguousarray(V.transpose(1, 0, 2)).astype(bf16)

    v0, v1, v2 = fold(W0), fold(W1), fold(W2)
    iota = np.broadcast_to(np.arange(WW, dtype=np.float32), (P, WW))
    identf = np.eye(P, dtype=np.float32)

    in_maps = []
    for c in range(CORES):
        rows = slice(c * SL, (c + 1) * SL)
        in_maps.append(dict(
            xfm=np.ascontiguousarray(x_pad[rows].T).astype(bf16),
            xtab=xtab.astype(bf16),
            idxA=pre["idxA"][c], idxB=pre["idxB"][c],
            dl=pre["dl"][c].astype(bf16),
            dinv=pre["dinv_nm"][c], dinv2=pre["dinv2_nm"][c],
            v0=v0, v1=v1, v2=v2,
            b0=np.asarray(b0, np.float32).reshape(HID, 1),
            b1=np.asarray(b1, np.float32).reshape(HID, 1),
            b2=np.asarray(b2, np.float32).reshape(CLS, 1),
            iota=np.ascontiguousarray(iota).astype(bf16),
            identb=identf.astype(bf16), identf=identf,
        ))

    nc = _get_nc(CA, CB, stage)
    kw = {}
    if trace:
        kw = dict(trace=True,
                  trace_cores=trace_cores if trace_cores is not None else [0])
    res = bass_utils.run_bass_kernel_spmd(nc, in_maps,
                                          core_ids=list(range(CORES)), **kw)

    full = np.concatenate([res.results[c]["out"] for c in range(CORES)],
                          axis=0)
    out = full[perm]  # row for old node i is at full[perm[i]]
    return out.astype(np.float32), res


def kernel(x, edge_src, edge_dst, W0, b0, W1, b1, W2, b2):
    out, _ = _run(x, edge_src, edge_dst, W0, b0, W1, b1, W2, b2)
    return out


# revision 12
# speedup vs baseline: 1.4075x; 1.3498x over previous
"""ChebConv GNN (3 layers, K=4) on 8 Trainium2 NeuronCores.

Monomial reformulation: Chebyshev polynomials are expanded so every hop is a
plain SpMM against the raw adjacency, u_{k+1} = A_hat (dinv^2 * u_k), with the
symmetric normalization folded into the node-major feature tables (dinv or
dinv^2 per row, applied with fused per-partition activation scales) and the
Chebyshev coefficients folded into the dense weights:
  out = x@V0 + diag(dinv) (u1@V1 + u2@V2 + u3@V3),
  V0 = W0-W2, V1 = 3W3-W1, V2 = 2W2, V3 = -4W3.
The scatter matrix per destination window is a pure one-hot built with a
single is_equal per window; the per-edge weight multiply disappears.

Sharding: nodes are partitioned across the 8 cores; destination nodes are
grouped into 128-wide windows (LPT-balanced on in-degree). Each hop gathers
source rows from a replicated node-major table in HBM (dma_gather, int16
indices, one call per (window, source-chunk), trailing -1 indices trimmed by
the gather ucode so padding costs no descriptors), segment-sums them with
one-hot matmuls on the TensorEngine into feature-major PSUM, and
re-replicates the per-core table slices with TWO half-table AllGathers per
hop (X = first half of every core's slice, issued mid-hop) so collectives
overlap gather descriptor generation.
"""

import numpy as np

# ---------------- problem constants (hardcoded per contract) ----------------
N, E = 50000, 800000
F, HID, CLS, K = 128, 128, 40, 4
P = 128
CORES = 8
NWIN = 50                # 128-wide dst windows per core
SL = NWIN * P            # 6400 nodes per core
CHK = SL // 2            # 3200: per-core rows per X/Y table chunk
NPAD = CORES * SL        # 51200 padded node count
HALF = NPAD // 2         # 25600 rows per chunk table (int16-indexable)
PF = 12                  # A-gather prefetch depth (windows)
PB = 2                   # B-gather prefetch depth


# ---------------- host preprocessing ----------------
def _lpt_windows(indeg, n_windows, cap):
    """Assign nodes to windows (cap nodes each), balancing in-degree sums.
    Returns perm: old node id -> new node id."""
    import heapq
    order = np.argsort(-indeg, kind="stable")
    heap = [(0, wi) for wi in range(n_windows)]
    heapq.heapify(heap)
    counts = np.zeros(n_windows, np.int64)
    perm = np.empty(len(indeg), np.int64)
    for old in order:
        while True:
            load, wi = heapq.heappop(heap)
            if counts[wi] < cap:
                break
        perm[old] = wi * cap + counts[wi]
        counts[wi] += 1
        if counts[wi] < cap:
            heapq.heappush(heap, (load + int(indeg[old]), wi))
    return perm


def _preprocess(edge_src, edge_dst, n):
    """Node permutation, per-core padded gather indices, and dl arrays."""
    es = np.asarray(edge_src, np.int64)
    ed = np.asarray(edge_dst, np.int64)
    deg = np.bincount(es, minlength=n).astype(np.float32)
    dinv = np.where(deg > 0, 1.0 / np.sqrt(np.maximum(deg, 1.0)), 0.0).astype(
        np.float32
    )

    indeg = np.bincount(ed, minlength=n)
    perm = _lpt_windows(indeg, CORES * NWIN, P)  # old -> new

    nsrc = perm[es]
    ndst = perm[ed]
    core_e = ndst // SL
    win_e = (ndst % SL) // P           # 0..NWIN-1
    dloc_e = (ndst % P).astype(np.float32)
    pos_e = nsrc % SL                  # position within src core
    chunk_e = (pos_e >= CHK).astype(np.int64)   # X=0 / Y=1 table chunk
    idx_e = ((nsrc // SL) * CHK + pos_e - chunk_e * CHK).astype(np.int64)

    # group edges by (core, win, chunk)
    gkey = (core_e * NWIN + win_e) * 2 + chunk_e
    ngroups = CORES * NWIN * 2
    order = np.argsort(gkey, kind="stable")
    gkey_s = gkey[order]
    counts = np.bincount(gkey_s, minlength=ngroups)
    starts = np.concatenate([[0], np.cumsum(counts)[:-1]])
    rank = np.arange(len(es)) - starts[gkey_s]  # position within group

    cnts = counts.reshape(CORES, NWIN, 2)
    CA = max(int(np.ceil(cnts[:, :, 0].max() / P)), 1)
    CB = max(int(np.ceil(cnts[:, :, 1].max() / P)), 1)

    capa = {0: CA * P, 1: CB * P}
    # idx slot arrays: pad with 0 (valid row; dl=-1 zeroes contribution)
    idx_pad = {h: np.zeros((CORES, NWIN, capa[h]), np.int16)
               for h in (0, 1)}
    CW = CA + CB
    dl_pad = np.full((CORES, NWIN, CW, P), -1.0, np.float32)

    ce, we, he = core_e[order], win_e[order], chunk_e[order]
    de, ie = dloc_e[order], idx_e[order]
    for h in (0, 1):
        m = he == h
        idx_pad[h][ce[m], we[m], rank[m]] = ie[m].astype(np.int16)
        boff = rank[m] // P + (0 if h == 0 else CA)
        dl_pad[ce[m], we[m], boff, rank[m] % P] = de[m]

    # dma_gather index arrays: one call per (window, chunk)
    def wrap(idxs):  # [cores, NWIN, L] -> [cores, 128, NWIN*(L//16)]
        c, g, L = idxs.shape
        a = idxs.reshape(c, g, L // 16, 16).transpose(0, 1, 3, 2)  # [c,g,16,L/16]
        a = np.tile(a, (1, 1, 8, 1))  # [c,g,128,L/16]
        return np.ascontiguousarray(a.transpose(0, 2, 1, 3).reshape(c, P, -1))

    idxA = wrap(idx_pad[0])
    idxB = wrap(idx_pad[1])

    # dl in SBUF layout [cores, 128(p), NWIN*CW]
    dl_arr = np.ascontiguousarray(
        dl_pad.transpose(0, 3, 1, 2).reshape(CORES, P, NWIN * CW))

    # node-major dinv packed per window: [cores, 128, NWIN]
    dn = np.zeros(NPAD, np.float32)
    dn[perm] = dinv
    dn_c = dn.reshape(CORES, NWIN, P)
    dinv_nm = np.ascontiguousarray(dn_c.transpose(0, 2, 1))
    dinv2_nm = np.ascontiguousarray((dn_c ** 2).transpose(0, 2, 1))

    return dict(perm=perm, dinv=dinv, CA=CA, CB=CB,
                idxA=idxA, idxB=idxB, dl=dl_arr,
                dinv_nm=dinv_nm, dinv2_nm=dinv2_nm)


# ---------------- device kernel ----------------
def _build(CA, CB, stage=99):
    import concourse.bass as bass
    import concourse.bacc as bacc
    import concourse.tile as tile
    import concourse.mybir as mybir
    import dataclasses

    CW = CA + CB
    NBLK = NWIN * CW
    fp = mybir.dt.float32
    bf = mybir.dt.bfloat16
    Alu = mybir.AluOpType
    Act = mybir.ActivationFunctionType

    nc = bacc.Bacc("TRN2", target_bir_lowering=False, debug=False,
                   num_devices=CORES, num_swdge_queues=4)

    # -------- I/O --------
    xfm_d = nc.dram_tensor("xfm", [P, SL], bf, kind="ExternalInput")
    xtabX_d = nc.dram_tensor("xtabX", [HALF, F], bf, kind="ExternalInput")
    xtabY_d = nc.dram_tensor("xtabY", [HALF, F], bf, kind="ExternalInput")
    idxA_d = nc.dram_tensor("idxA", [P, NWIN * CA * 8], mybir.dt.int16,
                            kind="ExternalInput")
    idxB_d = nc.dram_tensor("idxB", [P, NWIN * CB * 8], mybir.dt.int16,
                            kind="ExternalInput")
    dl_d = nc.dram_tensor("dl", [P, NBLK], bf, kind="ExternalInput")
    dinv_d = nc.dram_tensor("dinv", [P, NWIN], fp, kind="ExternalInput")
    dinv2_d = nc.dram_tensor("dinv2", [P, NWIN], fp, kind="ExternalInput")
    v0_d = nc.dram_tensor("v0", [P, K, HID], bf, kind="ExternalInput")
    v1_d = nc.dram_tensor("v1", [P, K, HID], bf, kind="ExternalInput")
    v2_d = nc.dram_tensor("v2", [P, K, CLS], bf, kind="ExternalInput")
    b0_d = nc.dram_tensor("b0", [HID, 1], fp, kind="ExternalInput")
    b1_d = nc.dram_tensor("b1", [HID, 1], fp, kind="ExternalInput")
    b2_d = nc.dram_tensor("b2", [CLS, 1], fp, kind="ExternalInput")
    iota_d = nc.dram_tensor("iota", [P, P], bf, kind="ExternalInput")
    identb_d = nc.dram_tensor("identb", [P, P], bf, kind="ExternalInput")
    identf_d = nc.dram_tensor("identf", [P, P], fp, kind="ExternalInput")
    out_d = nc.dram_tensor("out", [SL, CLS], fp, kind="ExternalOutput")

    def bmid(ap, n):  # [128, X] -> [128, n, X], middle stride 0
        return dataclasses.replace(ap, ap=[ap.ap[0], [0, n], ap.ap[1]])

    def blast(ap, n):  # [128, X] -> [128, X, n], last stride 0
        return dataclasses.replace(ap, ap=[ap.ap[0], ap.ap[1], [0, n]])

    with tile.TileContext(nc) as tc:
        with (
            tc.tile_pool(name="const", bufs=1) as constp,
            tc.tile_pool(name="h", bufs=1) as hp,
            tc.tile_pool(name="acc", bufs=1) as accp,
            tc.tile_pool(name="ga", bufs=PF + 2) as gap,
            tc.tile_pool(name="gb", bufs=PB + 2) as gbp,
            tc.tile_pool(name="m", bufs=3) as mp,
            tc.tile_pool(name="u", bufs=3) as up,
            tc.tile_pool(name="st", bufs=3) as stp,
            tc.tile_pool(name="ep", bufs=3) as epp,
            tc.tile_pool(name="psS", bufs=2, space="PSUM") as psS,
            tc.tile_pool(name="psW", bufs=2, space="PSUM") as psW,
            tc.tile_pool(name="psT", bufs=2, space="PSUM") as psT,
            tc.tile_pool(name="dram", bufs=2, space="DRAM") as dramp,
            tc.tile_pool(name="tabs", bufs=4, space="DRAM") as tabp,
        ):
            # -------- constants --------
            dl_t = constp.tile([P, NBLK], bf)
            dinv_t = constp.tile([P, NWIN], fp)
            dinv2_t = constp.tile([P, NWIN], fp)
            iota_t = constp.tile([P, P], bf)
            identb_t = constp.tile([P, P], bf)
            identf_t = constp.tile([P, P], fp)
            idxA_t = constp.tile([P, NWIN * CA * 8], mybir.dt.int16)
            idxB_t = constp.tile([P, NWIN * CB * 8], mybir.dt.int16)
            v0_t = constp.tile([P, K, HID], bf)
            v1_t = constp.tile([P, K, HID], bf)
            v2_t = constp.tile([P, K, CLS], bf)
            b0_t = constp.tile([HID, 1], fp)
            b1_t = constp.tile([HID, 1], fp)
            b2_t = constp.tile([CLS, 1], fp)
            for t, d in ((dl_t, dl_d), (dinv_t, dinv_d), (dinv2_t, dinv2_d),
                         (iota_t, iota_d), (identb_t, identb_d),
                         (identf_t, identf_d), (idxA_t, idxA_d),
                         (idxB_t, idxB_d), (v0_t, v0_d), (v1_t, v1_d),
                         (v2_t, v2_d), (b0_t, b0_d), (b1_t, b1_d),
                         (b2_t, b2_d)):
                nc.sync.dma_start(out=t[:], in_=d[:])

            h_fm = hp.tile([P, SL], bf, tag="hfm")
            nc.sync.dma_start(out=h_fm[:], in_=xfm_d[:, :])

            # zero-init gather buffers: stale tail slots (trimmed pads) must
            # be finite since M=0 kills them only if they aren't NaN
            for _ in range(PF + 2):
                GAz = gap.tile([P, CA, P], bf, tag="GA", name="GAz")
                nc.vector.memset(GAz[:], 0.0)
            for _ in range(PB + 2):
                GBz = gbp.tile([P, CB, P], bf, tag="GB", name="GBz")
                nc.vector.memset(GBz[:], 0.0)

            def allgather(slice_d):
                tab = tabp.tile([HALF, F], bf, tag="tab", addr_space="Shared")
                nc.gpsimd.collective_compute(
                    "AllGather", Alu.bypass,
                    replica_groups=[list(range(CORES))],
                    ins=[slice_d[:, :].opt()], outs=[tab[:, :].opt()])
                return tab

            def hop(tabX, tabY, Vt, k, fo, acc_b, want_table,
                    post_win=None):
                """u_k = A_hat @ table; acc_b (+)= u_k @ V_k.  want_table:
                write dinv^2-scaled node-major slices and allgather them
                (X chunk mid-loop, Y at the end); returns (tabX', tabY').
                post_win(w, u): fused per-window epilogue for hop 3."""
                sliceX = sliceY = tabXn = None
                if want_table:
                    sliceX = dramp.tile([CHK, F], bf, tag="sliceX",
                                        name="sliceX")
                    sliceY = dramp.tile([CHK, F], bf, tag="sliceY",
                                        name="sliceY")
                ga_tiles, gb_tiles = {}, {}
                for t in range(NWIN + PF):
                    if t < NWIN:
                        GA = gap.tile([P, CA, P], bf, tag="GA")
                        nc.gpsimd.dma_gather(
                            out_ap=GA[:], in_ap=tabX,
                            idxs_ap=idxA_t[:, t * CA * 8:(t + 1) * CA * 8],
                            num_idxs=CA * P, num_idxs_reg=CA * P,
                            elem_size=P, single_packet=False,
                            queue_num=(2 * t) % 4)
                        ga_tiles[t] = GA
                    tb = t - (PF - PB)
                    if 0 <= tb < NWIN:
                        GB = gbp.tile([P, CB, P], bf, tag="GB")
                        nc.gpsimd.dma_gather(
                            out_ap=GB[:], in_ap=tabY,
                            idxs_ap=idxB_t[:, tb * CB * 8:(tb + 1) * CB * 8],
                            num_idxs=CB * P, num_idxs_reg=CB * P,
                            elem_size=P, single_packet=False,
                            queue_num=(2 * tb + 1) % 4)
                        gb_tiles[tb] = GB
                    w = t - PF
                    if w < 0:
                        continue
                    GA, GB = ga_tiles.pop(w), gb_tiles.pop(w)
                    wb = slice(w * P, (w + 1) * P)
                    M = mp.tile([P, CW, P], bf, tag="M")
                    nc.vector.tensor_tensor(
                        out=M[:], in0=bmid(iota_t[:], CW),
                        in1=blast(dl_t[:, w * CW:(w + 1) * CW], P),
                        op=Alu.is_equal)
                    ps = psS.tile([P, P], fp, tag="ps")
                    for b in range(CW):
                        Gsl = GA[:, b, :] if b < CA else GB[:, b - CA, :]
                        nc.tensor.matmul(out=ps[:], lhsT=Gsl, rhs=M[:, b, :],
                                         start=(b == 0), stop=(b == CW - 1))
                    u = up.tile([P, P], bf, tag="u")
                    nc.scalar.activation(out=u[:], in_=ps[:], func=Act.Copy)
                    psw = psW.tile([P, P], fp, tag="psw")
                    nc.tensor.matmul(out=psw[:fo, :], lhsT=Vt[:, k, :fo],
                                     rhs=u[:], start=True, stop=True)
                    if k == 1:
                        nc.vector.tensor_copy(out=acc_b[:fo, wb],
                                              in_=psw[:fo, :])
                    else:
                        nc.vector.tensor_tensor(out=acc_b[:fo, wb],
                                                in0=acc_b[:fo, wb],
                                                in1=psw[:fo, :], op=Alu.add)
                    if want_table:
                        pst = psT.tile([P, P], bf, tag="pstb")
                        nc.tensor.transpose(out=pst[:], in_=u[:],
                                            identity=identb_t[:])
                        st = stp.tile([P, F], bf, tag="st")
                        nc.scalar.activation(out=st[:], in_=pst[:],
                                             func=Act.Copy,
                                             scale=dinv2_t[:, w:w + 1])
                        if w < NWIN // 2:
                            nc.sync.dma_start(
                                out=sliceX[w * P:(w + 1) * P, :], in_=st[:])
                            if w == NWIN // 2 - 1:
                                tabXn = allgather(sliceX)
                        else:
                            w2 = w - NWIN // 2
                            nc.sync.dma_start(
                                out=sliceY[w2 * P:(w2 + 1) * P, :], in_=st[:])
                    if post_win is not None:
                        post_win(w)
                if want_table:
                    return tabXn, allgather(sliceY)
                return None, None

            def make_epilogue(l, acc_a, acc_b, state):
                """Per-window epilogue closure for layer l, fused into hop 3.
                For layers 0/1 also writes the dinv-scaled h table slices and
                issues the X-chunk allgather mid-loop (into state)."""
                last = l == 2
                if not last:
                    hsX = dramp.tile([CHK, F], bf, tag="sliceX", name="hsX")
                    hsY = dramp.tile([CHK, F], bf, tag="sliceY", name="hsY")
                    state["hsY"] = hsY

                def post_win(w):
                    wb = slice(w * P, (w + 1) * P)
                    if not last:
                        pa = psT.tile([P, P], fp, tag="pst")
                        nc.tensor.transpose(out=pa[:], in_=acc_a[:, wb],
                                            identity=identf_t[:])
                        pbt = psT.tile([P, P], fp, tag="pst")
                        nc.tensor.transpose(out=pbt[:], in_=acc_b[:, wb],
                                            identity=identf_t[:])
                        hn = epp.tile([P, P], fp, tag="hn")
                        nc.vector.tensor_scalar(
                            out=hn[:], in0=pbt[:], scalar1=dinv_t[:, w:w + 1],
                            scalar2=None, op0=Alu.mult)
                        nc.vector.tensor_tensor(out=hn[:], in0=hn[:],
                                                in1=pa[:], op=Alu.add)
                        # table slice = dinv * relu(hn)
                        sth = stp.tile([P, F], bf, tag="st")
                        nc.scalar.activation(out=sth[:], in_=hn[:],
                                             func=Act.Relu,
                                             scale=dinv_t[:, w:w + 1])
                        if w < NWIN // 2:
                            nc.sync.dma_start(out=hsX[w * P:(w + 1) * P, :],
                                              in_=sth[:])
                            if w == NWIN // 2 - 1:
                                state["tabX"] = allgather(hsX)
                        else:
                            w2 = w - NWIN // 2
                            nc.sync.dma_start(out=hsY[w2 * P:(w2 + 1) * P, :],
                                              in_=sth[:])
                        # h (feature-major) = relu(hn) transposed back
                        hb = epp.tile([P, P], bf, tag="hb")
                        nc.scalar.activation(out=hb[:], in_=hn[:],
                                             func=Act.Relu)
                        ph = psT.tile([P, P], bf, tag="pstb")
                        nc.tensor.transpose(out=ph[:], in_=hb[:],
                                            identity=identb_t[:])
                        nc.scalar.activation(out=h_fm[:, wb], in_=ph[:],
                                             func=Act.Copy)
                    else:
                        pa = psT.tile([P, P], fp, tag="pst")
                        nc.tensor.transpose(out=pa[:, :CLS],
                                            in_=acc_a[:CLS, wb],
                                            identity=identf_t[:CLS, :CLS])
                        pbt = psT.tile([P, P], fp, tag="pst")
                        nc.tensor.transpose(out=pbt[:, :CLS],
                                            in_=acc_b[:CLS, wb],
                                            identity=identf_t[:CLS, :CLS])
                        t = epp.tile([P, CLS], fp, tag="t")
                        nc.vector.tensor_scalar(
                            out=t[:], in0=pbt[:, :CLS],
                            scalar1=dinv_t[:, w:w + 1],
                            scalar2=None, op0=Alu.mult)
                        nc.vector.tensor_tensor(out=t[:], in0=t[:],
                                                in1=pa[:, :CLS], op=Alu.add)
                        nm = epp.tile([P, 1], fp, tag="nm")
                        nc.vector.tensor_reduce(
                            out=nm[:], in_=t[:], op=Alu.max,
                            axis=mybir.AxisListType.X, negate=True)
                        ex = epp.tile([P, CLS], fp, tag="ex")
                        ssum = epp.tile([P, 1], fp, tag="ssum")
                        nc.scalar.activation(out=ex[:], in_=t[:],
                                             func=Act.Exp, bias=nm[:, 0:1],
                                             accum_out=ssum[:, 0:1])
                        lse = epp.tile([P, 1], fp, tag="lse")
                        nc.scalar.activation(out=lse[:], in_=ssum[:],
                                             func=Act.Ln)
                        res = epp.tile([P, CLS], fp, tag="res")
                        nc.vector.tensor_scalar(
                            out=res[:], in0=t[:],
                            scalar1=nm[:, 0:1], scalar2=lse[:, 0:1],
                            op0=Alu.add, op1=Alu.subtract)
                        nc.sync.dma_start(out=out_d[w * P:(w + 1) * P, :],
                                          in_=res[:])

                return post_win

            tabX_in, tabY_in = xtabX_d[:, :], xtabY_d[:, :]
            for l, (Vt, b_t, fo) in enumerate(
                    ((v0_t, b0_t, HID), (v1_t, b1_t, HID), (v2_t, b2_t, CLS))):
                if l * 10 >= stage:
                    break
                last = l == 2
                acc_a = accp.tile([P, SL], fp, tag="acca")
                acc_b = accp.tile([P, SL], fp, tag="accb")
                # ---- k=0 term: acc_a = V0.T @ h + b ----
                for w in range(NWIN):
                    wb = slice(w * P, (w + 1) * P)
                    psw = psW.tile([P, P], fp, tag="psw")
                    nc.tensor.matmul(out=psw[:fo, :], lhsT=Vt[:, 0, :fo],
                                     rhs=h_fm[:, wb], start=True, stop=True)
                    nc.scalar.activation(out=acc_a[:fo, wb], in_=psw[:fo, :],
                                         func=Act.Identity,
                                         bias=b_t[:fo, 0:1])
                # ---- hops 1..3 ----
                if stage < l * 10 + 2:
                    break
                tabX_in, tabY_in = hop(tabX_in, tabY_in, Vt, 1, fo, acc_b,
                                       True)
                if stage < l * 10 + 4:
                    break
                tabX_in, tabY_in = hop(tabX_in, tabY_in, Vt, 2, fo, acc_b,
                                       True)
                if stage < l * 10 + 6:
                    break
                state = {}
                post = make_epilogue(l, acc_a, acc_b, state)
                hop(tabX_in, tabY_in, Vt, 3, fo, acc_b, False, post_win=post)
                if not last:
                    tabX_in = state["tabX"]
                    tabY_in = allgather(state["hsY"])

    nc.compile()
    return nc


_CACHE = {}


def _get_nc(CA, CB, stage=99):
    key = (CA, CB, stage)
    if key not in _CACHE:
        _CACHE[key] = _build(CA, CB, stage)
    return _CACHE[key]


def _run(x, edge_src, edge_dst, W0, b0, W1, b1, W2, b2,
         trace=False, trace_cores=None, stage=99):
    from concourse import bass_utils

    import ml_dtypes
    bf16 = ml_dtypes.bfloat16

    n = x.shape[0]
    pre = _preprocess(edge_src, edge_dst, n)
    perm, dinv, CA, CB = pre["perm"], pre["dinv"], pre["CA"], pre["CB"]

    x = np.asarray(x, np.float32)
    x_pad = np.zeros((NPAD, F), np.float32)
    x_pad[perm] = x
    xtab = np.zeros((NPAD, F), np.float32)
    xtab[perm] = dinv[:, None] * x
    # split into X/Y chunks: X = rows [c*SL, c*SL+CHK) of every core c
    xt = xtab.reshape(CORES, 2, CHK, F)
    xtabX = np.ascontiguousarray(xt[:, 0].reshape(HALF, F)).astype(bf16)
    xtabY = np.ascontiguousarray(xt[:, 1].reshape(HALF, F)).astype(bf16)

    # folded monomial weights
    def fold(W):
        W = np.asarray(W, np.float32)
        V = np.stack([W[0] - W[2], 3.0 * W[3] - W[1], 2.0 * W[2],
                      -4.0 * W[3]])
        return np.ascontiguousarray(V.transpose(1, 0, 2)).astype(bf16)

    v0, v1, v2 = fold(W0), fold(W1), fold(W2)
    iota = np.broadcast_to(np.arange(P, dtype=np.float32), (P, P))
    identf = np.eye(P, dtype=np.float32)

    in_maps = []
    for c in range(CORES):
        rows = slice(c * SL, (c + 1) * SL)
        in_maps.append(dict(
            xfm=np.ascontiguousarray(x_pad[rows].T).astype(bf16),
            xtabX=xtabX, xtabY=xtabY,
            idxA=pre["idxA"][c], idxB=pre["idxB"][c],
            dl=pre["dl"][c].astype(bf16),
            dinv=pre["dinv_nm"][c], dinv2=pre["dinv2_nm"][c],
            v0=v0, v1=v1, v2=v2,
            b0=np.asarray(b0, np.float32).reshape(HID, 1),
            b1=np.asarray(b1, np.float32).reshape(HID, 1),
            b2=np.asarray(b2, np.float32).reshape(CLS, 1),
            iota=np.ascontiguousarray(iota).astype(bf16),
            identb=identf.astype(bf16), identf=identf,
        ))

    nc = _get_nc(CA, CB, stage)
    kw = {}
    if trace:
        kw = dict(trace=True,
                  trace_cores=trace_cores if trace_cores is not None else [0])
    res = bass_utils.run_bass_kernel_spmd(nc, in_maps,
                                          core_ids=list(range(CORES)), **kw)

    full = np.concatenate([res.results[c]["out"] for c in range(CORES)],
                          axis=0)
    out = full[perm]  # row for old node i is at full[perm[i]]
    return out.astype(np.float32), res


def kernel(x, edge_src, edge_dst, W0, b0, W1, b1, W2, b2):
    out, _ = _run(x, edge_src, edge_dst, W0, b0, W1, b1, W2, b2)
    return out
